# revision 7
# baseline (speedup 1.0000x reference)
"""Trainium2 Bass kernel for nn_LstmCloseModel (closed-loop LSTM over basins).

Data-parallel over the ngrid axis: 8 NeuronCores x 375 grid cells each,
replicated weights, full 365-step recurrence on-device per core.

Layout: feature-on-partition, grid-on-free.  State hT/cT live as [128,2,GP]
(H=256 split in two 128-partition chunks); gates are computed transposed
([4H, ngrid]) by PE matmuls with static weight tiles as the stationary
operand.  Default dtype bf16 (1 cycle/row on the PE, same as f32r, but
halves LDWEIGHTS time and SBUF traffic); set LSTM_MM_DT=f32r to fall back.

Closed-loop feedback restructure: the reference computes
  y_used(t) = observed ? y_obs : (wout.h(t-1) + bout)
            = y0(t) + mknot(t) * (wout.h(t-1) + bout)
with y0 = nan_to_num(y_obs)*observed and mknot = 1-observed, both known on
the host.  We fold y0 and mknot*bout into the static input matmul as two
extra K rows of xT (weights w_y and w_y*bout), so the only device-side
feedback work is ym = yo_psum * mknot (one vector op) followed by a K=2
matmul -- no copy_predicated / cast / fill chain.

PE stream per step (38 matmuls + WARM filler), ordered so the whh matmuls
cover the serial pred->ym and wy->relu chains, and warm matmuls bridge the
cell-state tail at the step boundary (keeps the PE p-state ramped):
  pred(2) | x0_static(2) | whh f,g,i-jb0 (10) | wy(2) | whh i-jb1(2) |
  wih f,g,i (12) | wave-o whh+wih (8) | warm(W)
Elementwise work is spread across engines: Scalar does the 8 gate
activations + 2 tanh(c); Vector does ym, both relu halves, f*c, i*g and
o*tanh(c); GpSimd does the c-accumulate adds and the output-row copy.
"""

import os
import sys
import types

sys.path.insert(0, "/opt/trn_rl_repo")

# NTFF profile hook (timing): the image's antenv package lacks axon_hooks;
# inject an equivalent so run_bass_kernel_spmd(trace=True) can measure HW time.
try:
    import antenv

    if not hasattr(antenv, "axon_hooks"):
        from trn_agent_boot.trn_boot import _ntff_profile_via_ctypes

        _hook = _ntff_profile_via_ctypes("/opt/axon/libaxon_pjrt.so")
        _mod = types.ModuleType("antenv.axon_hooks")
        _mod.get_axon_ntff_profile_hook = lambda: _hook
        _mod.set_axon_ntff_profile_hook = lambda h: None
        sys.modules["antenv.axon_hooks"] = _mod
        antenv.axon_hooks = _mod
except Exception:
    pass

import numpy as np

import concourse.bacc as bacc
import concourse.mybir as mybir
import concourse.tile as tile
from concourse.bass import ts
from concourse.bass_utils import run_bass_kernel_spmd

NT, NGRID, NX = 365, 3000, 20
H, NY = 256, 1
NCORES = 8
G = NGRID // NCORES       # 375 grid cells per core
GP = G + (G % 2)          # padded even for matmuls
KX = NX + 2               # x rows + y0 row + mknot row
F32 = mybir.dt.float32

_dt_env = os.environ.get("LSTM_MM_DT", "bf16")
MM_DT = {"f32r": mybir.dt.float32r, "bf16": mybir.dt.bfloat16, "f32": F32}[_dt_env]
WARM = int(os.environ.get("LSTM_WARM", "3"))

LAST_EXEC_NS = None
LAST_RESULTS = None


def build_nc():
    nc = bacc.Bacc("TRN2")

    xT_d = nc.declare_dram_parameter("xT", [NT, KX, GP], MM_DT, isOutput=False)
    mk_d = nc.declare_dram_parameter("mk32", [NT, GP], F32, isOutput=False)
    wih_d = nc.declare_dram_parameter("wihT", [128, 2, 4 * H], MM_DT, isOutput=False)
    whh_d = nc.declare_dram_parameter("whhT", [128, 2, 4 * H], MM_DT, isOutput=False)
    win_d = nc.declare_dram_parameter("winT", [KX, H], MM_DT, isOutput=False)
    wy2_d = nc.declare_dram_parameter("wy2T", [2, H], MM_DT, isOutput=False)
    wout_d = nc.declare_dram_parameter("woutT", [128, 2], MM_DT, isOutput=False)
    bg_d = nc.declare_dram_parameter("bg", [128, 8], F32, isOutput=False)
    bin_d = nc.declare_dram_parameter("bin", [128, 2], F32, isOutput=False)
    out_d = nc.declare_dram_parameter("outy", [NT, GP], F32, isOutput=True)

    AF = mybir.ActivationFunctionType
    OP = mybir.AluOpType

    with tile.TileContext(nc) as tc:
        with (
            tc.tile_pool(name="singles", bufs=1) as singles,
            tc.tile_pool(name="state", bufs=3) as state,
            tc.tile_pool(name="acts", bufs=3) as acts,
            tc.tile_pool(name="xio", bufs=6) as xio,
            tc.tile_pool(name="ps_x", bufs=1, space="PSUM") as ps_x,
            tc.tile_pool(name="ps_g", bufs=6, space="PSUM") as ps_g,
        ):
            # --- constants ---
            wih_s = singles.tile([128, 2, 4 * H], MM_DT)
            whh_s = singles.tile([128, 2, 4 * H], MM_DT)
            win_s = singles.tile([KX, H], MM_DT)
            wy2_s = singles.tile([2, H], MM_DT)
            wout_s = singles.tile([128, 2], MM_DT)
            bg_s = singles.tile([128, 8], F32)
            bin_s = singles.tile([128, 2], F32)
            ym_s = singles.tile([2, GP], MM_DT)  # row0 = ym, row1 = zeros
            nc.sync.dma_start(out=wih_s[:], in_=wih_d[:])
            nc.sync.dma_start(out=whh_s[:], in_=whh_d[:])
            nc.sync.dma_start(out=win_s[:], in_=win_d[:])
            nc.sync.dma_start(out=wy2_s[:], in_=wy2_d[:])
            nc.sync.dma_start(out=wout_s[:], in_=wout_d[:])
            nc.sync.dma_start(out=bg_s[:], in_=bg_d[:])
            nc.sync.dma_start(out=bin_s[:], in_=bin_d[:])
            nc.vector.memset(ym_s[:], 0.0)

            h_prev = None  # zero at t=0; h/c terms skipped then
            c_prev = None

            for t in range(NT):
                # ---- input DMAs (prefetched by pool depth)
                xt = xio.tile([KX, GP], MM_DT, tag="xt")
                nc.sync.dma_start(out=xt[:], in_=xT_d[t])
                if t > 0:
                    mk = xio.tile([1, GP], F32, tag="mk")
                    nc.sync.dma_start(out=mk[:], in_=mk_d[t : t + 1, :])

                # ---- x0 static part: K=22 (x rows + y0 + mknot*bout rows);
                # depends only on DMA -> emitted first to cover the wait for
                # h_{t-1} at the step boundary
                with nc.named_scope("x0"):
                    x0_ps = ps_x.tile([128, 2, 512], F32, tag="x0ps", name="x0_ps")
                    for jb in range(2):
                        nc.tensor.matmul(
                            x0_ps[:, jb, 0:GP], win_s[:, ts(jb, 128)], xt[:],
                            start=True, stop=(t == 0),
                        )

                # ---- pred from h_{t-1}; raw psum copied out (host adds b_out)
                if t > 0:
                    with nc.named_scope("pred"):
                        yo_ps = ps_g.tile([1, 512], F32, tag="g", name="yo_ps")
                        nc.tensor.matmul(
                            yo_ps[:, 0:GP], wout_s[:, 0:1], h_prev[:, 0, :],
                            start=True, stop=False,
                        )
                        nc.tensor.matmul(
                            yo_ps[:, 0:GP], wout_s[:, 1:2], h_prev[:, 1, :],
                            start=False, stop=True,
                        )
                        # ym = yo * mknot  (the only serial feedback op)
                        nc.vector.tensor_mul(ym_s[0:1, :], yo_ps[:, 0:GP], mk[:])
                        osb = xio.tile([1, GP], F32, tag="osb")
                        nc.vector.tensor_copy(osb[:], yo_ps[:, 0:GP])
                        nc.sync.dma_start(out=out_d[t - 1 : t, :], in_=osb[:])

                # ---- whh waves f,g,i (jb0 fully, then jb1 except wave i-jb1)
                g_pss = [
                    [ps_g.tile([128, 512], F32, tag="g", name=f"g{w}{jb}")
                     for jb in range(2)]
                    for w in range(3)
                ]
                if t > 0:
                    with nc.named_scope("whh"):
                        for k in range(2):
                            for w in range(3):
                                for jb in range(2):
                                    if w == 2 and jb == 1:
                                        continue  # moved after wy
                                    nc.tensor.matmul(
                                        g_pss[w][jb][:, 0:GP],
                                        whh_s[:, k, ts(2 * w + jb, 128)],
                                        h_prev[:, k, :],
                                        start=(k == 0), stop=False,
                                    )

                # ---- wy: x0 += w_y (x) ym   (K=2, row1 zero)
                if t > 0:
                    with nc.named_scope("wy"):
                        for jb in range(2):
                            nc.tensor.matmul(
                                x0_ps[:, jb, 0:GP], wy2_s[:, ts(jb, 128)],
                                ym_s[:], start=False, stop=True,
                            )
                    with nc.named_scope("whh"):
                        for k in range(2):
                            nc.tensor.matmul(
                                g_pss[2][1][:, 0:GP],
                                whh_s[:, k, ts(5, 128)],
                                h_prev[:, k, :],
                                start=(k == 0), stop=False,
                            )

                # ---- relu (both halves on vector; covered by whh i-jb1)
                with nc.named_scope("relu"):
                    x0_sb = acts.tile([128, 2, GP], MM_DT, tag="x0")
                    for jb in range(2):
                        nc.vector.tensor_scalar(
                            out=x0_sb[:, jb, :], in0=x0_ps[:, jb, 0:GP],
                            scalar1=bin_s[:, jb : jb + 1], scalar2=0.0,
                            op0=OP.add, op1=OP.max,
                        )

                # ---- gate waves f,g,i (wih) with cell chain interleaved
                c_new = state.tile([128, 2, GP], F32, tag="c")
                h_new = state.tile([128, 2, GP], MM_DT, tag="h")
                tc_t = acts.tile([128, 2, GP], MM_DT, tag="tanh_c")
                fc = acts.tile([128, 2, GP], F32, tag="fc")
                ig = acts.tile([128, 2, GP], F32, tag="ig")
                gact = []
                for w in range(3):
                    with nc.named_scope(f"wave{w}"):
                        a_sb = acts.tile([128, 2, GP], MM_DT, tag=f"act{w}")
                        for jb in range(2):
                            col = ts(2 * w + jb, 128)
                            for k in range(2):
                                nc.tensor.matmul(
                                    g_pss[w][jb][:, 0:GP], wih_s[:, k, col],
                                    x0_sb[:, k, :],
                                    start=(t == 0 and k == 0),
                                    stop=(k == 1),
                                )
                            nc.scalar.activation(
                                out=a_sb[:, jb, :], in_=g_pss[w][jb][:, 0:GP],
                                func=AF.Tanh if w == 1 else AF.Sigmoid,
                                bias=bg_s[:, 2 * w + jb : 2 * w + jb + 1],
                            )
                        gact.append(a_sb)
                    if w == 0 and t > 0:
                        with nc.named_scope("cell"):
                            for jb in range(2):
                                nc.vector.tensor_mul(
                                    fc[:, jb, :],
                                    gact[0][:, jb, :], c_prev[:, jb, :],
                                )
                    if w == 2:
                        with nc.named_scope("cell"):
                            for jb in range(2):
                                if t > 0:
                                    nc.vector.tensor_mul(
                                        ig[:, jb, :],
                                        gact[2][:, jb, :], gact[1][:, jb, :],
                                    )
                                    nc.gpsimd.tensor_add(
                                        c_new[:, jb, :],
                                        fc[:, jb, :], ig[:, jb, :],
                                    )
                                else:
                                    nc.vector.tensor_mul(
                                        c_new[:, jb, :],
                                        gact[2][:, jb, :], gact[1][:, jb, :],
                                    )
                                nc.scalar.activation(
                                    out=tc_t[:, jb, :], in_=c_new[:, jb, :],
                                    func=AF.Tanh,
                                )

                # ---- wave o + h per H-half
                with nc.named_scope("waveo"):
                    so = acts.tile([128, 2, GP], MM_DT, tag="act3")
                    for jb in range(2):
                        g_ps = ps_g.tile([128, 512], F32, tag="g",
                                         name=f"g3{jb}")
                        col = ts(6 + jb, 128)
                        if t > 0:
                            for k in range(2):
                                nc.tensor.matmul(
                                    g_ps[:, 0:GP], whh_s[:, k, col],
                                    h_prev[:, k, :], start=(k == 0), stop=False,
                                )
                        for k in range(2):
                            nc.tensor.matmul(
                                g_ps[:, 0:GP], wih_s[:, k, col],
                                x0_sb[:, k, :],
                                start=(t == 0 and k == 0), stop=(k == 1),
                            )
                        nc.scalar.activation(
                            out=so[:, jb, :], in_=g_ps[:, 0:GP],
                            func=AF.Sigmoid,
                            bias=bg_s[:, 6 + jb : 6 + jb + 1],
                        )
                        nc.vector.tensor_mul(
                            h_new[:, jb, :], so[:, jb, :], tc_t[:, jb, :]
                        )

                if WARM > 0:
                    with nc.named_scope("warm"):
                        dmy = ps_g.tile([128, 512], F32, tag="g", name="dmy")
                        for d in range(WARM):
                            nc.tensor.matmul(
                                dmy[:, 0:GP], whh_s[:, 0, ts(d, 128)],
                                x0_sb[:, 0, :], start=True, stop=True,
                            )

                h_prev, c_prev = h_new, c_new

            # final output row from h_{NT-1}
            with nc.named_scope("pred"):
                yo_ps = ps_g.tile([1, 512], F32, tag="g", name="yo_ps")
                nc.tensor.matmul(
                    yo_ps[:, 0:GP], wout_s[:, 0:1], h_prev[:, 0, :],
                    start=True, stop=False,
                )
                nc.tensor.matmul(
                    yo_ps[:, 0:GP], wout_s[:, 1:2], h_prev[:, 1, :],
                    start=False, stop=True,
                )
                osb = xio.tile([1, GP], F32, tag="osb")
                nc.vector.tensor_copy(osb[:], yo_ps[:, 0:GP])
                nc.sync.dma_start(out=out_d[NT - 1 : NT, :], in_=osb[:])

    nc.finalize()
    return nc


def kernel(x, y, w_in, b_in, w_ih, b_ih, w_hh, b_hh, w_out, b_out):
    global LAST_EXEC_NS, LAST_RESULTS
    x = np.asarray(x, np.float32)
    y = np.asarray(y, np.float32)

    # gate reorder [i,f,g,o] -> wave order [f,g,i,o]
    perm = np.concatenate(
        [np.arange(H, 2 * H), np.arange(2 * H, 3 * H), np.arange(0, H),
         np.arange(3 * H, 4 * H)]
    )
    wih_r = np.asarray(w_ih, np.float32)[perm]          # [1024, 256]
    whh_r = np.asarray(w_hh, np.float32)[perm]
    bg_r = (np.asarray(b_ih, np.float32) + np.asarray(b_hh, np.float32))[perm]

    wih_dev = np.ascontiguousarray(
        wih_r.T.reshape(2, 128, 4 * H).transpose(1, 0, 2))  # [128,2,1024]
    whh_dev = np.ascontiguousarray(
        whh_r.T.reshape(2, 128, 4 * H).transpose(1, 0, 2))
    bg_dev = np.ascontiguousarray(bg_r.reshape(8, 128).T)   # [128,8]

    # winT covers xT rows: 20 x rows, y0 row (weight w_y), mknot row
    # (weight w_y*bout); wy2T row0 = w_y (row1 zero) for the ym matmul
    w_in = np.asarray(w_in, np.float32)                      # [256, 21]
    w_y = w_in[:, NX]                                        # [256]
    bout_f = float(np.asarray(b_out).reshape(-1)[0])
    win_re = np.concatenate(
        [w_in[:, :NX], w_y[:, None], (w_y * bout_f)[:, None]], axis=1)
    win_dev = np.ascontiguousarray(win_re.T)                 # [22, 256]
    wy2_dev = np.ascontiguousarray(np.stack(
        [w_y, np.zeros(H, np.float32)]))                     # [2, 256]
    bin_dev = np.ascontiguousarray(
        np.asarray(b_in, np.float32).reshape(2, 128).T)      # [128,2]

    wout_dev = np.ascontiguousarray(
        np.asarray(w_out, np.float32).reshape(2, 128).T)     # [128,2]

    y2 = y[:, :, 0]                                          # [NT, NGRID]
    obs = ~np.isnan(y2)
    y0_full = np.where(obs, np.nan_to_num(y2, nan=0.0), 0.0).astype(np.float32)
    mknot_full = (~obs).astype(np.float32)                   # 1 where missing

    if MM_DT == mybir.dt.bfloat16:
        import ml_dtypes
        cast = lambda a: np.asarray(a).astype(ml_dtypes.bfloat16)
    else:
        cast = lambda a: a
    wih_dev, whh_dev, win_dev, wout_dev, wy2_dev = map(
        cast, (wih_dev, whh_dev, win_dev, wout_dev, wy2_dev))
    nc = build_nc()
    in_maps = []
    for c in range(NCORES):
        g0, g1 = c * G, (c + 1) * G
        xT = np.zeros((NT, KX, GP), np.float32)
        xT[:, :NX, :G] = x[:, g0:g1, :].transpose(0, 2, 1)
        xT[:, NX, :G] = y0_full[:, g0:g1]
        xT[1:, NX + 1, :G] = mknot_full[1:, g0:g1]  # t=0: no pred feedback
        xT = cast(xT)
        mk32 = np.zeros((NT, GP), np.float32)
        mk32[:, :G] = mknot_full[:, g0:g1]
        in_maps.append(
            {
                "xT": xT, "mk32": mk32,
                "wihT": wih_dev, "whhT": whh_dev, "winT": win_dev,
                "wy2T": wy2_dev, "woutT": wout_dev, "bg": bg_dev,
                "bin": bin_dev,
            }
        )

    res = None
    for attempt in range(3):
        try:
            res = run_bass_kernel_spmd(nc, in_maps, core_ids=list(range(NCORES)))
            break
        except Exception:
            if attempt == 2:
                raise
    LAST_EXEC_NS = res.exec_time_ns
    LAST_RESULTS = res

    out = np.empty((NT, NGRID, NY), np.float32)
    for c in range(NCORES):
        out[:, c * G : (c + 1) * G, 0] = res.results[c]["outy"][:, :G] + bout_f
    return out
